# revision 27
# baseline (speedup 1.0000x reference)
"""GRU kernel for 8x Trainium2 NeuronCores (Bass/Tile).

Problem: T=512, B=64, IN=512, HID=1024, OUT=512 GRU:
    z = sigmoid(h @ W_zh + x_t @ W_zx + b_z)
    r = sigmoid(h @ W_rh + x_t @ W_rx + b_r)
    c = tanh((h*r) @ W_hh + x_t @ W_hx + b_h)
    h = (1-z)*h + z*c ;  y_t = h @ W_out + b_out
Returns (Y[T*B, OUT], h_final[B, HID]).

Strategy: data-parallel over batch (B=64 -> 8 per core).
  Phase 1 (bulk):  G = x @ [W_zx|W_rx|W_hx] + b   for all (t,b) -> DRAM
  Phase 2 (seq):   512 recurrence steps; state kept transposed h^T[hid, b]
                   so matmul out = W[k,m].T @ h^T[k,:] accumulates in PSUM
                   with hid_out on partitions.  Weights bf16 (FWL), PSUM fp32.
  Phase 3 (bulk):  Y = H^T.T @ W_out  (lhsT = stored h^T tiles)
b_out is added on host.
"""

import numpy as np
import ml_dtypes

import concourse.bass as bass
import concourse.bacc as bacc
import concourse.mybir as mybir
from concourse.bass import ts, _add_dep_helper
from concourse.tile import TileContext
from concourse.bass_utils import run_bass_kernel_spmd

T, B, IN, HID, OUT = 512, 64, 512, 1024, 512
NCORES = 8
BL = B // NCORES          # 8 batch rows per core
TB = T * BL               # 4096
P = 128
KH = HID // P             # 8 hid k/m tiles
KI = IN // P              # 4 input k tiles
F32 = mybir.dt.float32
BF16 = mybir.dt.bfloat16
AF = mybir.ActivationFunctionType
BF = ml_dtypes.bfloat16


def build_nc(n_steps=T):
    nc = bacc.Bacc(None, target_bir_lowering=False)

    # ---- I/O (per-core values supplied by host) ----
    xT = nc.declare_dram_parameter("xT", [KI, P, TB], BF16, isOutput=False)
    h0T = nc.declare_dram_parameter("h0T", [P, KH * BL], F32, isOutput=False)
    h0Tb = nc.declare_dram_parameter("h0Tb", [P, KH * BL], BF16, isOutput=False)
    Wzh = nc.declare_dram_parameter("Wzh", [P, KH * HID], BF16, isOutput=False)
    Wrh = nc.declare_dram_parameter("Wrh", [P, KH * HID], BF16, isOutput=False)
    Whh = nc.declare_dram_parameter("Whh", [P, KH * HID], BF16, isOutput=False)
    Wzx = nc.declare_dram_parameter("Wzx", [P, KI * HID], BF16, isOutput=False)
    Wrx = nc.declare_dram_parameter("Wrx", [P, KI * HID], BF16, isOutput=False)
    Whx = nc.declare_dram_parameter("Whx", [P, KI * HID], BF16, isOutput=False)
    Wout = nc.declare_dram_parameter("Wout", [P, KH * OUT], BF16, isOutput=False)
    bzrh = nc.declare_dram_parameter("bzrh", [P, 3 * KH], F32, isOutput=False)
    Y = nc.declare_dram_parameter("Y", [OUT // P, P, TB], F32, isOutput=True)
    hTfin = nc.declare_dram_parameter("hTfin", [P, KH * BL], F32, isOutput=True)

    # ---- DRAM scratch ----
    # G[g, p, m, tb]: gate-g preactivation contribution from x (+bias),
    # in transposed layout (partition = hid slice within tile m).
    G = nc.dram_tensor("G", [3, P, KH, TB], F32)

    nch_total = (n_steps * BL + 511) // 512  # 512-wide tb chunks needed

    with TileContext(nc) as tc:
        with (
            tc.tile_pool(name="const", bufs=1) as cpool,
            tc.tile_pool(name="state", bufs=1) as spool,
            tc.tile_pool(name="work", bufs=4) as wk,
            tc.tile_pool(name="gbuf", bufs=6) as gpool,
            tc.tile_pool(name="ps_big", bufs=2, space="PSUM") as ps1,
            tc.tile_pool(name="ps_rec", bufs=2, space="PSUM") as ps2,
        ):
            # ---- load constants / state ----
            wzh = cpool.tile([P, KH * HID], BF16)
            wrh = cpool.tile([P, KH * HID], BF16)
            whh = cpool.tile([P, KH * HID], BF16)
            nc.sync.dma_start(out=wzh[:], in_=Wzh[:])
            nc.sync.dma_start(out=wrh[:], in_=Wrh[:])
            nc.sync.dma_start(out=whh[:], in_=Whh[:])
            wx = []
            for nm, src in (("wzx", Wzx), ("wrx", Wrx), ("whx", Whx)):
                t_ = cpool.tile([P, KI * HID], BF16, tag=nm)
                nc.sync.dma_start(out=t_[:], in_=src[:])
                wx.append(t_)
            wout = cpool.tile([P, KH * OUT], BF16)
            nc.sync.dma_start(out=wout[:], in_=Wout[:])
            bsb = cpool.tile([P, 3 * KH], F32)
            nc.sync.dma_start(out=bsb[:], in_=bzrh[:])
            hT = spool.tile([P, KH * BL], F32)
            nc.sync.dma_start(out=hT[:], in_=h0T[:])
            # h^T history slab (bf16), one KH block per hid k-tile; column 0
            # of each block holds h0, step t writes columns (t+1)*BL.
            # The recurrence reads its rhs and phase 3 reads its lhsT tiles
            # directly from here (no DRAM round trip, no in-place WAR).
            HTB = (n_steps + 1) * BL
            hh = cpool.tile([P, KH * HTB], BF16)
            hh3 = hh[:].rearrange("p (k c) -> p k c", k=KH)
            nc.sync.dma_start(out=hh3[:, :, 0:BL], in_=h0Tb[:])

            def hcol(k, slot):
                return hh[:, k * HTB + slot * BL : k * HTB + (slot + 1) * BL]

            # ---- phase 1: G = x @ Wx + b (all t,b) ----
            gw = {}  # (nch, g) -> last G-write DMA of that chunk
            for nch in range(nch_total):
                xk = []
                for k in range(KI):
                    xt_ = gpool.tile([P, 512], BF16, tag=f"xs{k}")
                    nc.sync.dma_start(
                        out=xt_[:], in_=xT[k, :, nch * 512 : (nch + 1) * 512]
                    )
                    xk.append(xt_)
                for g in range(3):
                    for m in range(KH):
                        ps = ps1.tile([P, 512], F32, tag="psb")
                        for k in range(KI):
                            nc.tensor.matmul(
                                ps[:],
                                wx[g][:, (k * KH + m) * P : (k * KH + m + 1) * P],
                                xk[k][:],
                                start=(k == 0),
                                stop=(k == KI - 1),
                            )
                        ct = wk.tile([P, 512], F32, tag="gcp")
                        nc.scalar.activation(
                            ct[:], ps[:], AF.Identity,
                            bias=bsb[:, g * KH + m : g * KH + m + 1],
                        )
                        gw[(nch, g)] = nc.sync.dma_start(
                            out=G[g, :, m, nch * 512 : (nch + 1) * 512], in_=ct[:]
                        )

            # ---- phase 2: recurrence ----
            # Gate preactivations (x-part + bias) are DMA'd from DRAM straight
            # into the PSUM banks; the recurrent matmuls then accumulate on
            # top with start=False.  This removes the three serializing
            # DVE adds from the per-step critical path.
            HALF = KH // 2  # m-tiles per half for the candidate tail split

            PFD = 4  # DRAM->SBUF gate prefetch depth (steps)
            gtiles = {}

            def fetch_g(t):
                # Tile does not reliably order DMA reads after DMA writes
                # through DRAM scratch (different HW-DGE queues), so pin the
                # read of each G chunk behind that chunk's last write.
                tl = []
                for g, tag in ((1, "gr"), (0, "gz"), (2, "gc")):
                    gt = gpool.tile([P, KH * BL], F32, tag=tag)
                    rd = nc.sync.dma_start(out=gt[:], in_=G[g, :, :, ts(t, BL)])
                    _add_dep_helper(rd.ins, gw[(t * BL // 512, g)].ins, reason="G raw")
                    tl.append(gt)
                gtiles[t] = tl

            psum_q = {}

            def preload(t):
                # SBUF -> PSUM bias preload on ScalarE (off critical path)
                tl = []
                for gt, tag in zip(gtiles.pop(t), ("psr", "psz", "psc")):
                    pt = ps2.tile([P, KH * BL], F32, tag=tag)
                    nc.scalar.copy(out=pt[:], in_=gt[:])
                    tl.append(pt)
                psum_q[t] = tl

            hwr = {}  # t -> last history-slab write (half 1 CAST) of step t
            for t in range(min(PFD, n_steps)):
                fetch_g(t)
            preload(0)
            if n_steps > 1:
                preload(1)

            for t in range(n_steps):
                if t + PFD < n_steps:
                    fetch_g(t + PFD)
                if t + 2 < n_steps:
                    preload(t + 2)
                psr, psz, psc = psum_q.pop(t)

                # r gate matmuls, k-outer: k-group reads only h slot-t cols of
                # k, so step t can begin while step t-1's later blends run.
                for k in range(KH):
                    for m in range(KH):
                        nc.tensor.matmul(
                            psr[:, ts(m, BL)],
                            wrh[:, (k * KH + m) * P : (k * KH + m + 1) * P],
                            hcol(k, t),
                            start=False,
                            stop=(k == KH - 1),
                            skip_group_check=True,
                        )
                rr = wk.tile([P, KH * BL], F32, tag="rr")
                nc.scalar.activation(rr[:], psr[:], AF.Sigmoid)
                hrb = wk.tile([P, KH * BL], BF16, tag="hrb")
                nc.vector.tensor_mul(hrb[:], hT[:], rr[:])

                # z gate matmuls (PE continues while ACT/DVE do r's tail)
                for k in range(KH):
                    for m in range(KH):
                        nc.tensor.matmul(
                            psz[:, ts(m, BL)],
                            wzh[:, (k * KH + m) * P : (k * KH + m + 1) * P],
                            hcol(k, t),
                            start=False,
                            stop=(k == KH - 1),
                            skip_group_check=True,
                        )
                zz = wk.tile([P, KH * BL], F32, tag="zz")
                nc.scalar.activation(zz[:], psz[:], AF.Sigmoid)

                # candidate matmuls in two m-halves; each half's tanh+blend
                # overlaps the other half's matmuls.
                cc = wk.tile([P, KH * BL], F32, tag="cc")
                for half in range(2):
                    for m in range(half * HALF, (half + 1) * HALF):
                        for k in range(KH):
                            nc.tensor.matmul(
                                psc[:, ts(m, BL)],
                                whh[:, (k * KH + m) * P : (k * KH + m + 1) * P],
                                hrb[:, ts(k, BL)],
                                start=False,
                                stop=(k == KH - 1),
                                skip_group_check=True,
                            )
                    hs = slice(half * HALF * BL, (half + 1) * HALF * BL)
                    ks = slice(half * HALF, (half + 1) * HALF)
                    nc.scalar.activation(cc[:, hs], psc[:, hs], AF.Tanh)
                    nc.vector.tensor_sub(cc[:, hs], cc[:, hs], hT[:, hs])
                    nc.vector.tensor_mul(cc[:, hs], zz[:, hs], cc[:, hs])
                    nc.vector.tensor_add(hT[:, hs], hT[:, hs], cc[:, hs])
                    hT3 = hT[:].rearrange("p (k b) -> p k b", k=KH)
                    hwr[t] = nc.vector.tensor_copy(
                        hh3[:, ks, (t + 1) * BL : (t + 2) * BL], hT3[:, ks, :]
                    )

            # ---- phase 3: Y^T = W_out.T @ H^T ----
            # W_out tiles are the stationary operand (static, no race with the
            # PE LDWEIGHTS pull-ahead); the freshly written history slab is the
            # moving operand, whose reads are ordered like the recurrence rhs.
            rows3 = n_steps * BL
            for mo in range(OUT // P):
                for nch in range((rows3 + 511) // 512):
                    w3 = min(512, rows3 - nch * 512)
                    psy = ps1.tile([P, 512], F32, tag="psb")
                    for k in range(KH):
                        mm = nc.tensor.matmul(
                            psy[:, 0:w3],
                            wout[:, k * OUT + mo * P : k * OUT + (mo + 1) * P],
                            hh[:, k * HTB + BL + nch * 512 : k * HTB + BL + nch * 512 + w3],
                            start=(k == 0),
                            stop=(k == KH - 1),
                        )
                        if k == 0:
                            _add_dep_helper(
                                mm.ins,
                                hwr[min(nch * 64 + 63, n_steps - 1)].ins,
                                reason="H raw",
                            )
                    yt = wk.tile([P, 512], F32, tag="yt")
                    nc.vector.tensor_copy(yt[:, 0:w3], psy[:, 0:w3])
                    nc.sync.dma_start(
                        out=Y[mo, :, nch * 512 : nch * 512 + w3], in_=yt[:, 0:w3]
                    )

            nc.sync.dma_start(out=hTfin[:], in_=hT[:])

    nc.compile()
    return nc


def _prep_weights(W_zh, W_zx, b_z, W_rh, W_rx, b_r, W_hh, W_hx, b_h, W_out):
    def wh(w):  # [HID, HID] -> [128, KH*HID], col=(k,m,j)
        return np.ascontiguousarray(
            w.reshape(KH, P, KH, P).transpose(1, 0, 2, 3).reshape(P, KH * HID)
        ).astype(BF)

    def wxp(w):  # [IN, HID] -> [128, KI*HID]
        return np.ascontiguousarray(
            w.reshape(KI, P, KH, P).transpose(1, 0, 2, 3).reshape(P, KI * HID)
        ).astype(BF)

    wo = np.ascontiguousarray(
        W_out.reshape(KH, P, OUT).transpose(1, 0, 2).reshape(P, KH * OUT)
    ).astype(BF)
    bz = np.stack([b_z, b_r, b_h]).reshape(3, KH, P).transpose(2, 0, 1)
    bz = np.ascontiguousarray(bz.reshape(P, 3 * KH)).astype(np.float32)
    return {
        "Wzh": wh(W_zh), "Wrh": wh(W_rh), "Whh": wh(W_hh),
        "Wzx": wxp(W_zx), "Wrx": wxp(W_rx), "Whx": wxp(W_hx),
        "Wout": wo, "bzrh": bz,
    }


def _run(nc, inputs, n_steps=T, trace=False, trace_kwargs=None):
    x, h0 = inputs["x"], inputs["h0"]
    wmap = _prep_weights(
        inputs["W_zh"], inputs["W_zx"], inputs["b_z"],
        inputs["W_rh"], inputs["W_rx"], inputs["b_r"],
        inputs["W_hh"], inputs["W_hx"], inputs["b_h"], inputs["W_out"],
    )
    in_maps = []
    for c in range(NCORES):
        xc = x[:, c * BL : (c + 1) * BL, :]  # [T, BL, IN]
        xTc = np.ascontiguousarray(
            xc.transpose(2, 0, 1).reshape(KI, P, TB)
        ).astype(BF)
        h0c = h0[c * BL : (c + 1) * BL]  # [BL, HID]
        h0Tc = np.ascontiguousarray(
            h0c.T.reshape(KH, P, BL).transpose(1, 0, 2).reshape(P, KH * BL)
        ).astype(np.float32)
        m = dict(wmap)
        m["xT"] = xTc
        m["h0T"] = h0Tc
        m["h0Tb"] = h0Tc.astype(BF)
        in_maps.append(m)

    kw = dict(trace_kwargs or {})
    res = run_bass_kernel_spmd(nc, in_maps, list(range(NCORES)), trace=trace, **kw)

    ntb_rows = n_steps * BL
    Ys = []
    hfin = np.zeros((B, HID), np.float32)
    for c in range(NCORES):
        out = res.results[c]
        yt = np.asarray(out["Y"], np.float32)  # [OUT//P, P, TB] = Y^T tiles
        yc = yt.reshape(OUT, yt.shape[2]).T  # [TB, OUT]
        Ys.append(yc[:ntb_rows].reshape(n_steps, BL, OUT))
        hTf = np.asarray(out["hTfin"], np.float32)
        hfin[c * BL : (c + 1) * BL] = (
            hTf.reshape(P, KH, BL).transpose(2, 1, 0).reshape(BL, HID)
        )
    Yfull = np.stack(Ys, axis=1).reshape(n_steps * B, OUT)
    Yfull = Yfull + inputs["b_out"][None, :].astype(np.float32)
    return (Yfull, hfin), res


def kernel(**inputs):
    nc = build_nc(T)
    (Yfull, hfin), _ = _run(nc, inputs, T)
    return Yfull.astype(np.float32), hfin.astype(np.float32)
